# revision 7
# baseline (speedup 1.0000x reference)
"""Bidirectional LSTM encoder (batch 1, T=2048, I=H=1024) on Trainium2.

Strategy: fwd and rev directions are independent -> run the SAME single-core
Bass program on 2 NeuronCores (SPMD), feeding core 1 the time-reversed input
and the reverse-direction weights.  Host numpy does layout prep / unprep.

Per core:
  phase 1: input projection xg = x @ Wih.T + (bih+bhh) as a bf16 PE GEMM,
           streamed to DRAM in per-step "spread" layout.
  phase 2: 2048 sequential LSTM steps.  Per step the 4096x1024 matvec
           Whh @ h runs as 4 concurrent PE column-groups (tile_position),
           h-chunk columns as stationary operands (M=1).  Gate rows land in
           PSUM partitions {0,32,64,96}; one DVE StreamTranspose spreads
           them across all 128 partitions; sigmoid/tanh on ACT and the c/h
           update on DVE run as quadrant-aligned [32,32] tiles.
"""

import sys

try:
    import concourse  # noqa: F401  (provided by the axon site PYTHONPATH)
except ImportError:
    sys.path.insert(0, "/opt/trn_rl_repo")

import numpy as np
import ml_dtypes

import concourse.bass as bass
import concourse.tile as tile
from concourse import bacc
from concourse import mybir

F32 = mybir.dt.float32
BF16 = mybir.dt.bfloat16
AF = mybir.ActivationFunctionType

I = 1024
H = 1024
G = 4096
KC = 8
NB = 16  # steps per loop body


def build_lstm_nc(T, debug=False, warm_dummies=2):
    assert T % NB == 0
    nc = bacc.Bacc("TRN2", target_bir_lowering=False, debug=debug)

    whh_d = nc.dram_tensor("whh_t", [128, KC, G], BF16, kind="ExternalInput")
    wih_d = nc.dram_tensor("wih_t", [128, KC, G], BF16, kind="ExternalInput")
    xt_d = nc.dram_tensor("xt", [128, KC, T], BF16, kind="ExternalInput")
    bias_d = nc.dram_tensor("bias_p", [1, G], BF16, kind="ExternalInput")
    hst_d = nc.dram_tensor("hst", [T, 128, 8], F32, kind="ExternalOutput")
    cfin_d = nc.dram_tensor("cfin", [128, 8], F32, kind="ExternalOutput")
    xgt_d = nc.dram_tensor("xgt", [T + NB, 128, 32], F32)

    MSZ = min(128, T)
    n_mchunks = (T + MSZ - 1) // MSZ

    with tile.TileContext(nc) as tc:
        from contextlib import ExitStack

        with ExitStack() as ctx:
            consts = ctx.enter_context(tc.tile_pool(name="consts", bufs=1))
            whh_sb = consts.tile([128, KC * G], BF16)
            nc.sync.dma_start(out=whh_sb[:], in_=whh_d.ap().rearrange("p k g -> p (k g)"))

            # Recurrence pools/tiles allocated up-front so their SBUF/PSUM
            # addresses never overlap freed GEMM-phase space (address reuse
            # would pile WAR waits onto single instructions).
            rec = ctx.enter_context(tc.tile_pool(name="rec", bufs=1))
            tout_pool = ctx.enter_context(tc.tile_pool(name="tout", bufs=2))
            gc_pool = ctx.enter_context(tc.tile_pool(name="gc", bufs=2))
            tmp_pool = ctx.enter_context(tc.tile_pool(name="tmp", bufs=2))
            mv_pool = ctx.enter_context(tc.tile_pool(name="mvps", bufs=1, space="PSUM"))

            h_sl = [rec.tile([128, 8], BF16, name=f"h{i}") for i in range(2)]
            c_sl = [rec.tile([128, 8], F32, name=f"c{i}") for i in range(2)]
            ax = rec.tile([128, NB * 32], F32)
            slab = rec.tile([128, NB * 8], F32)
            mv = [mv_pool.tile([128, 1024], F32, name=f"mv{i}") for i in range(2)]
            dps = mv_pool.tile([1, 512], F32)

            for sl in (0, 1):
                nc.vector.memset(h_sl[sl][:], 0.0)
                nc.vector.memset(c_sl[sl][:], 0.0)
                nc.vector.memset(mv[sl][:], 0.0)
            # pin loop-pool slot addresses before the GEMM phase allocates
            for _pool, _shape, _tag in (
                (tout_pool, [128, 1024], "tout"),
                (gc_pool, [128, 32], "gc"),
                (tmp_pool, [128, 8], "t1"),
                (tmp_pool, [128, 8], "t2"),
                (tmp_pool, [128, 8], "t3"),
            ):
                _pin = _pool.tile(_shape, F32, tag=_tag, name=f"pin_{_tag}")
                nc.vector.memset(_pin[:], 0.0)

            with ExitStack() as gctx:
                gpool = gctx.enter_context(tc.tile_pool(name="gemm", bufs=1))
                xt_pool = gctx.enter_context(tc.tile_pool(name="xtc", bufs=2))
                st_pool = gctx.enter_context(tc.tile_pool(name="stage", bufs=2))
                gps_pool = gctx.enter_context(
                    tc.tile_pool(name="gpsum", bufs=2, space="PSUM")
                )
                wih_sb = gpool.tile([128, KC * G], BF16)
                nc.sync.dma_start(
                    out=wih_sb[:], in_=wih_d.ap().rearrange("p k g -> p (k g)")
                )
                bias_sb = gpool.tile([1, G], BF16)
                nc.sync.dma_start(out=bias_sb[:], in_=bias_d.ap())
                ones_sb = gpool.tile([1, 128], BF16)
                nc.vector.memset(ones_sb[:], 1.0)

                for m in range(n_mchunks):
                    xtc = xt_pool.tile([128, KC * MSZ], BF16, tag="xtc")
                    nc.sync.dma_start(
                        out=xtc[:],
                        in_=xt_d.ap()[:, :, m * MSZ : (m + 1) * MSZ],
                    )
                    stage = st_pool.tile([128, G], F32, tag="stage")
                    for gq in range(8):
                        pg = gps_pool.tile([128, 512], F32, tag="gps")
                        for kc in range(KC):
                            nc.tensor.matmul(
                                pg[:MSZ, :],
                                lhsT=xtc[:, kc * MSZ : kc * MSZ + MSZ],
                                rhs=wih_sb[:, kc * G + 512 * gq : kc * G + 512 * (gq + 1)],
                                start=(kc == 0),
                                stop=False,
                            )
                        nc.tensor.matmul(
                            pg[:MSZ, :],
                            lhsT=ones_sb[0:1, :MSZ],
                            rhs=bias_sb[0:1, 512 * gq : 512 * (gq + 1)],
                            start=False,
                            stop=True,
                        )
                        nc.scalar.copy(stage[:MSZ, 512 * gq : 512 * (gq + 1)], pg[:MSZ, :])
                    nc.sync.dma_start(
                        out=xgt_d.ap()[m * MSZ : m * MSZ + MSZ],
                        in_=stage[:MSZ, :],
                    )

            with tc.For_i(
                0, T, NB,
                hint_engines=(mybir.EngineType.PE, mybir.EngineType.DVE),
                staggered_reset=True,
            ) as iv:
                nc.sync.dma_start(
                    out=ax[:],
                    in_=xgt_d.ap()[bass.ds(iv, NB)].transpose([1, 0, 2]),
                )
                for u in range(NB):
                    wr, rd = u & 1, (u + 1) & 1
                    h_rd, h_wr = h_sl[rd], h_sl[wr]
                    c_rd, c_wr = c_sl[rd], c_sl[wr]
                    pt = mv[u & 1]
                    for kc in range(KC):
                        for g in range(4):
                            for nh in range(2):
                                nc.tensor.matmul(
                                    pt[32 * g : 32 * g + 1, 512 * nh : 512 * (nh + 1)],
                                    lhsT=h_rd[:, kc : kc + 1],
                                    rhs=whh_sb[
                                        :,
                                        kc * G + 1024 * g + 512 * nh : kc * G
                                        + 1024 * g
                                        + 512 * (nh + 1),
                                    ],
                                    start=(kc == 0),
                                    stop=(kc == KC - 1),
                                    tile_position=(0, 32 * g),
                                )
                    for _ in range(warm_dummies):
                        nc.tensor.matmul(
                            dps[0:1, :],
                            lhsT=h_rd[:, 0:1],
                            rhs=whh_sb[:, 0:512],
                            start=True,
                            stop=True,
                            tile_position=(0, 0),
                        )
                    tout = tout_pool.tile([128, 1024], F32, tag="tout")
                    nc.vector.transpose(tout[:], pt[:])
                    gc = gc_pool.tile([128, 32], F32, tag="gc")
                    nc.vector.tensor_add(
                        gc[:], tout[:, 0 : 1024 : 32], ax[:, u * 32 : (u + 1) * 32]
                    )
                    # gate blocks at cols 8*b+s: i=0:8, f=8:16, g=16:24, o=24:32
                    nc.scalar.activation(gc[:, 0:16], gc[:, 0:16], AF.Sigmoid)
                    nc.scalar.activation(gc[:, 16:24], gc[:, 16:24], AF.Tanh)
                    nc.scalar.activation(gc[:, 24:32], gc[:, 24:32], AF.Sigmoid)
                    t1 = tmp_pool.tile([128, 8], F32, tag="t1")
                    t2 = tmp_pool.tile([128, 8], F32, tag="t2")
                    t3 = tmp_pool.tile([128, 8], F32, tag="t3")
                    nc.vector.tensor_mul(t1[:], gc[:, 8:16], c_rd[:])
                    nc.vector.tensor_mul(t2[:], gc[:, 0:8], gc[:, 16:24])
                    nc.vector.tensor_add(c_wr[:], t1[:], t2[:])
                    nc.scalar.activation(t3[:], c_wr[:], AF.Tanh)
                    nc.vector.tensor_mul(h_wr[:], gc[:, 24:32], t3[:])
                    nc.vector.tensor_copy(
                        out=slab[:, u * 8 : (u + 1) * 8], in_=h_wr[:]
                    )
                nc.sync.dma_start(
                    out=hst_d.ap()[bass.ds(iv, NB)].transpose([1, 0, 2]),
                    in_=slab[:],
                )

            nc.sync.dma_start(out=cfin_d.ap(), in_=c_sl[1][:])

    nc.finalize()
    return nc


# spread-unit map: hidden unit at partition p, col s is UA[p, s]
_P = np.arange(128)
UA = (256 * (_P[:, None] // 32) + (_P[:, None] % 32) + 32 * np.arange(8)[None, :])
UAF = UA.reshape(-1)  # [1024]


def prep_inputs(x, Wih, Whh, bih, bhh):
    T = x.shape[0]
    bf = ml_dtypes.bfloat16
    # whh_t[p, kc, 1024*g + 256*b + 32*sh + i] = Whh[1024*b + 256*g + 32*sh + i, UA[p, kc]]
    Wk = Whh[:, UAF].reshape(4, 4, 8, 32, 128, 8)  # [b, g, sh, i, p, kc]
    whh_t = np.ascontiguousarray(
        Wk.transpose(4, 5, 1, 0, 2, 3).reshape(128, KC, G)
    ).astype(bf)
    # GEMM out col g'' = 32*p' + 8*b + sh  <->  Wih row 1024*b + UA[p', sh]
    W2 = Wih.reshape(4, 4, 8, 32, I).transpose(1, 3, 0, 2, 4).reshape(G, I)
    wih_t = np.ascontiguousarray(
        W2.reshape(G, KC, 128).transpose(2, 1, 0)
    ).astype(bf)
    b = (bih + bhh).astype(np.float32)
    bias_p = np.ascontiguousarray(
        b.reshape(4, 4, 8, 32).transpose(1, 3, 0, 2).reshape(1, G)
    ).astype(bf)
    xt = np.ascontiguousarray(x.reshape(T, KC, 128).transpose(2, 1, 0)).astype(bf)
    return {"whh_t": whh_t, "wih_t": wih_t, "xt": xt, "bias_p": bias_p}


def post_outputs(hst, cfin):
    T = hst.shape[0]
    hs = np.empty((T, H), np.float32)
    hs[:, UAF] = hst.reshape(T, H)
    h_fin = hs[-1].copy()
    c_fin = np.empty(H, np.float32)
    c_fin[UAF] = cfin.reshape(H)
    return hs, h_fin, c_fin


_CACHE = {}


def _get_nc(T):
    if T not in _CACHE:
        _CACHE[T] = build_lstm_nc(T)
    return _CACHE[T]


def run_device(x, fwd_Wih, fwd_Whh, fwd_bih, fwd_bhh,
               rev_Wih, rev_Whh, rev_bih, rev_bhh, **spmd_kwargs):
    """Returns (outputs, hidden, cell, BassKernelResults)."""
    from concourse.bass_utils import run_bass_kernel_spmd

    x = np.asarray(x, dtype=np.float32)
    T = x.shape[0]
    nc = _get_nc(T)
    in_fwd = prep_inputs(x, np.asarray(fwd_Wih, np.float32),
                         np.asarray(fwd_Whh, np.float32),
                         np.asarray(fwd_bih, np.float32),
                         np.asarray(fwd_bhh, np.float32))
    in_rev = prep_inputs(np.ascontiguousarray(x[::-1]),
                         np.asarray(rev_Wih, np.float32),
                         np.asarray(rev_Whh, np.float32),
                         np.asarray(rev_bih, np.float32),
                         np.asarray(rev_bhh, np.float32))
    br = run_bass_kernel_spmd(nc, [in_fwd, in_rev], core_ids=[0, 1], **spmd_kwargs)
    hs_f, hf_f, cf_f = post_outputs(
        np.asarray(br.results[0]["hst"]), np.asarray(br.results[0]["cfin"])
    )
    hs_rs, hf_r, cf_r = post_outputs(
        np.asarray(br.results[1]["hst"]), np.asarray(br.results[1]["cfin"])
    )
    rev_hs = hs_rs[::-1]
    outputs = np.concatenate([hs_f, rev_hs], axis=1).astype(np.float32)
    hidden = np.concatenate([hf_f, hf_r])[None, None, :].astype(np.float32)
    cell = np.concatenate([cf_f, cf_r])[None, None, :].astype(np.float32)
    return outputs, hidden, cell, br


def kernel(x, fwd_Wih, fwd_Whh, fwd_bih, fwd_bhh,
           rev_Wih, rev_Whh, rev_bih, rev_bhh):
    outputs, hidden, cell, _ = run_device(
        x, fwd_Wih, fwd_Whh, fwd_bih, fwd_bhh,
        rev_Wih, rev_Whh, rev_bih, rev_bhh,
    )
    return outputs, hidden, cell
